# revision 46
# baseline (speedup 1.0000x reference)
"""GridEncoder (instant-NGP hash grid) forward on 8 Trainium2 NeuronCores.

Strategy (point-sharded SPMD):
  - Each core processes a 32768-point slice of input_means over all 16 levels.
  - Per level, the embedding table is staged in SBUF as bf16; within each
    16-partition group, partition q holds a contiguous slice of the level
    table.  Every group holds the full level table, so the 8 Q7 cores gather
    independent index streams.
  - DVE computes cell coords, corner hashes (idx), per-corner trilinear
    weights; idx splits into (hi = partition, off = row-in-partition).
  - gpsimd.ap_gather fetches, for each index, the candidate rows from all 16
    partitions of the group; a weight-premultiplied one-hot mask (hi == q)
    zeroes the 15 wrong candidates.  ap_gather cost is per-INDEX (~30ns,
    independent of d), so dense levels store the table twice (even- and
    odd-shifted row pairs) and fetch BOTH x-corners with one d=4 index:
    pair_idx = (row >> 1) + (row & 1) * (hsize/2).
  - (hi, w) per index is broadcast to the group's 16 partitions via a DRAM
    round-trip in bf16; the one-hot mask costs 2 DVE ops, and the j-order
    permutation is folded into a strided AP on the mask multiply (free).
  - TensorE reduces the 16 partitions of each group with a fixed 128x8
    block-ones matrix, accumulating all corner contributions into PSUM.
  - Table staging is the main serial overhead (no SBUF room to double-buffer
    a 128KB/partition table), so 8 hashed levels run LEVEL-PARALLEL ("phase
    A"): each 16-partition group holds a DIFFERENT level's table (staged once
    for all 8) and processes ALL of the core's points for its level.  The
    hash math is identical across hashed levels; only `scale` differs and is
    supplied as a per-partition vector (pvec).  This collapses 8 staging
    stalls into one.
"""
import math
import sys

sys.path.insert(0, "/opt/trn_rl_repo")

import numpy as np
import ml_dtypes

from concourse.bass import AP
from concourse.bacc import Bacc
import concourse.mybir as mybir
from concourse.tile import TileContext
from concourse import bass_utils

# ---- problem constants (hardcoded from the nn_GridEncoder problem) ----
NUM_LEVEL = 16
BASE_RES = 16
LOG2_T = 19
LEVEL_SCALE = 1.38191288
N_POINTS = 262144
P1 = 2654435761
P2 = 805459861

NCORES = 8
NPC = N_POINTS // NCORES          # 32768 points per core
NG = NPC // 8                     # 4096 points per 16-partition group
NB = 2048                         # points per group per batch
SB = NB // 16                     # 128 slots per partition per batch
NBATCH = NG // NB                 # 2

F32 = mybir.dt.float32
I32 = mybir.dt.int32
I16 = mybir.dt.int16
BF16 = mybir.dt.bfloat16
F16 = mybir.dt.float16
Op = mybir.AluOpType


def _grid_meta():
    max_len = 2 ** LOG2_T
    offs = []
    off = 0
    for i in range(NUM_LEVEL):
        res = int(np.ceil(BASE_RES * LEVEL_SCALE ** i))
        p = min(max_len, res ** 3)
        p = int(np.ceil(p / 8) * 8)
        offs.append(off)
        off += p
    offs.append(off)
    return offs


def _levels():
    offs = _grid_meta()
    lg = math.log2(LEVEL_SCALE)
    lv = []
    for l in range(NUM_LEVEL):
        hsize = offs[l + 1] - offs[l]
        scale = 2.0 ** (l * lg) * BASE_RES - 1.0
        res = int(math.ceil(scale)) + 1
        hashed = res ** 3 > hsize
        d = dict(l=l, off=offs[l], hsize=hsize, scale=scale, res=res,
                 hashed=hashed)
        if hashed:
            chunk = 1 << max(0, (hsize + 15) // 16 - 1).bit_length()
            while chunk * 16 < hsize:
                chunk <<= 1
            d["chunk"] = chunk
            d["lc"] = chunk.bit_length() - 1
        else:
            # pair mode: table stored twice (A: rows 2j,2j+1; B: rows
            # 2j+1,2j+2), interleaved in 8-PAIR blocks: pair P lives at
            # partition (P>>3)&15, slot ((P>>7)<<3)|(P&7).  8-pair blocks
            # keep staging source reads at 64B bursts AND writes 16-wide.
            # B region starts at P = SAB (128-pair aligned):
            # idx = (row>>1) + (row&1)*SAB.
            half = hsize // 2
            nj = (half + 127) // 128          # 128-pair staging rows per copy
            d["nj"] = nj
            d["sab"] = 128 * nj
            d["ne"] = 16 * nj                 # pair slots per partition
            d["half"] = half
        lv.append(d)
    return lv


LEVELS = _levels()
import os as _os
_LSEL = _os.environ.get("KLEVELS")
if _LSEL:
    _sel = [int(x) for x in _LSEL.split(",")]
    LEVELS = [lv for lv in LEVELS if lv["l"] in _sel]
    PA_LEVELS = []          # level-parallel phase disabled under KLEVELS
    OLD_LEVELS = LEVELS
else:
    # phase A: 8 consecutive hashed levels, one per 16-partition group
    # (identical hash math; only `scale` differs -> per-partition vector)
    PA_LEVELS = [lv for lv in LEVELS if lv["l"] in range(5, 13)]
    OLD_LEVELS = [lv for lv in LEVELS if lv["l"] not in range(5, 13)]
KREP = int(_os.environ.get("KREPEAT", "1"))
EMB_ROWS = _grid_meta()[-1]

_NC_CACHE = None


def _build():
    nc = Bacc("TRN2", target_bir_lowering=False)
    means = nc.dram_tensor("means", [NPC, 3], F32, kind="ExternalInput")
    emb = nc.dram_tensor("emb", [EMB_ROWS, 2], BF16, kind="ExternalInput")
    smat = nc.dram_tensor("smat", [128, 8], BF16, kind="ExternalInput")
    qvec = nc.dram_tensor("qvec", [128, 1], F32, kind="ExternalInput")
    pvec = nc.dram_tensor("pvec", [128, 1], F32, kind="ExternalInput")
    out = nc.dram_tensor("out", [NPC, 32], F16, kind="ExternalOutput")

    corners = [((c >> 0) & 1, (c >> 1) & 1, (c >> 2) & 1) for c in range(8)]

    HB = SB // 2  # slots per partition per dense half-call (64)
    corners4 = [(ky, kz) for kz in (0, 1) for ky in (0, 1)]

    with TileContext(nc) as tc:
        with tc.tile_pool(name="persist", bufs=1) as pp, \
             tc.tile_pool(name="tab", bufs=1) as tabp, \
             tc.tile_pool(name="work", bufs=1) as wp, \
             tc.tile_pool(name="io", bufs=2) as iw, \
             tc.tile_pool(name="gath", bufs=2) as gp, \
             tc.tile_pool(name="bcast", bufs=2) as bcp, \
             tc.tile_pool(name="amask", bufs=2) as amp, \
             tc.tile_pool(name="ps", bufs=1, space="PSUM") as psp, \
             tc.tile_pool(name="scr", bufs=2, space="DRAM") as dp:

            # persistent: means in slot-major layout; partition 16g+q slot s
            # holds point g*NG + s*16 + q
            means_t = pp.tile([128, NG // 16, 3], F32)
            for g in range(8):
                m_ap = AP(means[:].tensor, g * NG * 3,
                          [[3, 16], [48, NG // 16], [1, 3]])
                nc.sync.dma_start(out=means_t[16 * g:16 * (g + 1)], in_=m_ap)
            smat_t = pp.tile([128, 8], BF16)
            nc.sync.dma_start(out=smat_t[:], in_=smat[:])
            qv = pp.tile([128, 1], F32)
            nc.sync.dma_start(out=qv[:], in_=qvec[:])
            qvb = pp.tile([128, 1], BF16)
            nc.vector.tensor_copy(out=qvb[:], in_=qv[:])
            pv = pp.tile([128, 1], F32)
            nc.sync.dma_start(out=pv[:], in_=pvec[:])

            def stage_flat(tab4, g, cap, dst0, src0, n):
                # copy emb flat elems [src0, src0+n) into group g's partitions
                # (cap elems each), starting at flat in-group offset dst0
                p, foff, src, rem = dst0 // cap, dst0 % cap, src0, n
                while rem > 0:
                    if foff == 0 and rem >= cap:
                        nfull = rem // cap
                        nc.sync.dma_start(
                            out=tab4[16 * g + p:16 * g + p + nfull]
                                .rearrange("p a b -> p (a b)"),
                            in_=AP(emb[:].tensor, src, [[cap, nfull], [1, cap]]))
                        p += nfull
                        src += nfull * cap
                        rem -= nfull * cap
                        continue
                    take = min(cap - foff, rem)
                    nc.sync.dma_start(
                        out=tab4[16 * g + p:16 * g + p + 1,
                                 foff // 4:(foff + take) // 4, :]
                            .rearrange("p a b -> p (a b)"),
                        in_=AP(emb[:].tensor, src, [[1, take]]))
                    src += take
                    rem -= take
                    p += 1
                    foff = 0

            def stage(LV):
                hsize = LV["hsize"]
                if LV["hashed"]:
                    chunk = LV["chunk"]
                    tab = tabp.tile([128, chunk, 2], BF16, tag="tab")
                    nfull, rem = hsize // chunk, hsize % chunk
                    if nfull + (1 if rem else 0) < 16:
                        nc.vector.memset(tab[:], 0.0)
                    for g in range(8):
                        p0 = 16 * g
                        if nfull:
                            src = AP(emb[:].tensor, LV["off"] * 2,
                                     [[chunk * 2, nfull], [1, chunk * 2]])
                            nc.sync.dma_start(
                                out=tab[p0:p0 + nfull].rearrange("p a b -> p (a b)"),
                                in_=src)
                        if rem:
                            src = AP(emb[:].tensor, (LV["off"] + nfull * chunk) * 2,
                                     [[1, rem * 2]])
                            nc.sync.dma_start(
                                out=tab[p0 + nfull:p0 + nfull + 1, 0:rem]
                                    .rearrange("p a b -> p (a b)"),
                                in_=src)
                    return tab
                # dense pair mode, 8-pair-block interleave.  Each copy stages
                # as ONE full-width DMA per group with 64B source runs; the
                # padded tail reads a little past the level's rows (valid emb
                # memory, masked out by the one-hot select).
                nj, ne = LV["nj"], LV["ne"]
                tab = tabp.tile([128, ne, 4], BF16, tag="tab")
                for g in range(8):
                    for slot0, src0 in ((0, LV["off"] * 2),
                                        (8 * nj, LV["off"] * 2 + 2)):
                        nc.sync.dma_start(
                            out=tab[16 * g:16 * (g + 1), slot0:slot0 + 8 * nj, :],
                            in_=AP(emb[:].tensor, src0,
                                   [[32, 16], [512, nj], [1, 32]]))
                return tab

            def prep(LV, b, pa=False):
                hashed = LV["hashed"]
                pos = wp.tile([128, SB, 3], F32, tag="pos")
                if pa:
                    # phase A: every group processes the same point batch for
                    # its own level; per-group scale comes from pvec
                    means_b = iw.tile([128, SB, 3], F32, tag="mb")
                    for g in range(8):
                        nc.sync.dma_start(
                            out=means_b[16 * g:16 * (g + 1)],
                            in_=AP(means[:].tensor, b * NB * 3,
                                   [[3, 16], [48, SB], [1, 3]]))
                    nc.vector.tensor_scalar(out=pos[:], in0=means_b[:], scalar1=1.0,
                                            scalar2=0.5, op0=Op.add, op1=Op.mult)
                    pva = pv[:]
                    sc_bc = AP(pva.tensor, pva.offset, [list(pva.ap[0]), [0, SB], [0, 3]])
                    nc.vector.tensor_tensor(out=pos[:], in0=pos[:], in1=sc_bc, op=Op.mult)
                else:
                    msl = means_t[:, b * SB:(b + 1) * SB, :]
                    # pos = ((x+1)*0.5) * scale   (match reference fp order)
                    nc.vector.tensor_scalar(out=pos[:], in0=msl, scalar1=1.0,
                                            scalar2=0.5, op0=Op.add, op1=Op.mult)
                    nc.vector.tensor_single_scalar(
                        out=pos[:], in_=pos[:],
                        scalar=float(np.float32(LV["scale"])), op=Op.mult)
                # floor robust to cast rounding mode
                pgi = wp.tile([128, SB, 3], I32, tag="pgi")
                pgf = wp.tile([128, SB, 3], F32, tag="pgf")
                gtt = wp.tile([128, SB, 3], F32, tag="gtt")
                nc.vector.tensor_copy(out=pgi[:], in_=pos[:])
                nc.vector.tensor_copy(out=pgf[:], in_=pgi[:])
                nc.vector.tensor_tensor(out=gtt[:], in0=pgf[:], in1=pos[:], op=Op.is_gt)
                nc.vector.tensor_tensor(out=pgf[:], in0=pgf[:], in1=gtt[:], op=Op.subtract)
                nc.vector.tensor_copy(out=pgi[:], in_=pgf[:])
                frac = wp.tile([128, SB, 3], F32, tag="frac")
                omf = wp.tile([128, SB, 3], F32, tag="omf")
                nc.vector.tensor_tensor(out=frac[:], in0=pos[:], in1=pgf[:], op=Op.subtract)
                nc.vector.tensor_scalar(out=omf[:], in0=frac[:], scalar1=-1.0,
                                        scalar2=1.0, op0=Op.mult, op1=Op.add)
                wx = [omf[:, :, 0], frac[:, :, 0]]
                wy = [omf[:, :, 1], frac[:, :, 1]]
                wz = [omf[:, :, 2], frac[:, :, 2]]
                t1 = wp.tile([128, SB], I32, tag="t1")
                t2 = wp.tile([128, SB], I32, tag="t2")

                if not hashed:
                    r = LV["res"]
                    sab = LV["sab"]
                    ay0 = wp.tile([128, SB], I32, tag="c10")
                    ay1 = wp.tile([128, SB], I32, tag="c11")
                    az0 = wp.tile([128, SB], I32, tag="c20")
                    az1 = wp.tile([128, SB], I32, tag="c21")
                    nc.vector.tensor_single_scalar(out=ay0[:], in_=pgi[:, :, 1], scalar=r, op=Op.mult)
                    nc.vector.tensor_single_scalar(out=ay1[:], in_=ay0[:], scalar=r, op=Op.add)
                    nc.vector.tensor_single_scalar(out=az0[:], in_=pgi[:, :, 2], scalar=r * r, op=Op.mult)
                    nc.vector.tensor_single_scalar(out=az1[:], in_=az0[:], scalar=r * r, op=Op.add)
                    ay = [ay0[:], ay1[:]]
                    az = [az0[:], az1[:]]
                    wyz = {}
                    for i in range(2):
                        for j in range(2):
                            t = wp.tile([128, SB], F32, tag=f"wxy{i}{j}")
                            nc.vector.tensor_tensor(out=t[:], in0=wy[i], in1=wz[j], op=Op.mult)
                            wyz[(i, j)] = t[:]
                    off_d = iw.tile([128, 4, SB], I16, tag="off")
                    hiw = iw.tile([128, 4, SB, 3], BF16, tag="hiw")
                    tU = wp.tile([128, SB], I32, tag="tU")
                    tV = wp.tile([128, SB], I32, tag="yv")
                    for j, (ky, kz) in enumerate(corners4):
                        nc.vector.tensor_tensor(out=t1[:], in0=pgi[:, :, 0], in1=ay[ky], op=Op.add)
                        nc.vector.tensor_tensor(out=t2[:], in0=t1[:], in1=az[kz], op=Op.add)
                        # pair index P = (row >> 1) + (row & 1) * SAB;
                        # 8-pair-block interleave: hi = (P >> 3) & 15,
                        # off = ((P >> 7) << 3) | (P & 7)
                        nc.vector.tensor_single_scalar(out=tU[:], in_=t2[:], scalar=1, op=Op.logical_shift_right)
                        nc.vector.tensor_single_scalar(out=tV[:], in_=t2[:], scalar=1, op=Op.bitwise_and)
                        nc.vector.scalar_tensor_tensor(out=t2[:], in0=tV[:], scalar=sab,
                                                       in1=tU[:], op0=Op.mult, op1=Op.add)
                        nc.vector.tensor_single_scalar(out=tU[:], in_=t2[:], scalar=7, op=Op.logical_shift_right)
                        nc.vector.tensor_single_scalar(out=tU[:], in_=tU[:], scalar=3, op=Op.logical_shift_left)
                        nc.vector.tensor_single_scalar(out=tV[:], in_=t2[:], scalar=7, op=Op.bitwise_and)
                        nc.vector.tensor_tensor(out=t1[:], in0=tU[:], in1=tV[:], op=Op.bitwise_or)
                        nc.vector.tensor_copy(out=off_d[:, j, :], in_=t1[:])
                        nc.vector.tensor_scalar(out=t1[:], in0=t2[:], scalar1=3,
                                                scalar2=15, op0=Op.logical_shift_right, op1=Op.bitwise_and)
                        nc.vector.tensor_copy(out=hiw[:, j, :, 0], in_=t1[:])
                        nc.vector.tensor_tensor(out=hiw[:, j, :, 1], in0=wyz[(ky, kz)], in1=wx[0], op=Op.mult)
                        nc.vector.tensor_tensor(out=hiw[:, j, :, 2], in0=wyz[(ky, kz)], in1=wx[1], op=Op.mult)
                    # round-trip to DRAM: [g][j][h][q][s_l][t]  (h = half);
                    # the write is issued later (prep_flush) so it can't
                    # head-of-line-block the current batch's bcast reads
                    scr = dp.tile([8, 4, 2, 16, SB // 2, 3], BF16, tag="scr")
                    return off_d, scr, hiw, "dense"

                chunk, lc = LV["chunk"], LV["lc"]
                ax1 = wp.tile([128, SB], I32, tag="ax1")
                nc.vector.tensor_single_scalar(out=ax1[:], in_=pgi[:, :, 0], scalar=1, op=Op.add)
                ax = [pgi[:, :, 0], ax1[:]]
                ay = [None, None]
                az = [None, None]
                tmpm = wp.tile([128, SB], I32, tag="tmpm")
                for (arr, axis, mm) in ((ay, 1, P1), (az, 2, P2)):
                    t0 = wp.tile([128, SB], I32, tag=f"c{axis}0")
                    t1a = wp.tile([128, SB], I32, tag=f"c{axis}1")
                    # DVE int32 mult saturates and tensor ADD is f32-rounded:
                    # multiply by (prime & 0x7FFFF) split at bit 13 with
                    # carry-free recombination (adds stay < 2^18).
                    mmod = mm & 0x7FFFF
                    blo, ahi = mmod & 0x1FFF, mmod >> 13
                    tU = wp.tile([128, SB], I32, tag="tU")
                    yv = wp.tile([128, SB], I32, tag="yv")
                    nc.vector.tensor_copy(out=yv[:], in_=pgi[:, :, axis])
                    for tout in (t0, t1a):
                        nc.vector.tensor_single_scalar(out=tU[:], in_=yv[:], scalar=blo, op=Op.mult)
                        nc.vector.tensor_single_scalar(out=tmpm[:], in_=tU[:], scalar=13, op=Op.logical_shift_right)
                        nc.vector.tensor_single_scalar(out=tout[:], in_=yv[:], scalar=ahi, op=Op.mult)
                        nc.vector.tensor_tensor(out=tout[:], in0=tout[:], in1=tmpm[:], op=Op.add)
                        nc.vector.tensor_single_scalar(out=tout[:], in_=tout[:], scalar=13, op=Op.logical_shift_left)
                        nc.vector.tensor_single_scalar(out=tU[:], in_=tU[:], scalar=0x1FFF, op=Op.bitwise_and)
                        nc.vector.tensor_tensor(out=tout[:], in0=tout[:], in1=tU[:], op=Op.bitwise_or)
                        nc.vector.tensor_single_scalar(out=yv[:], in_=yv[:], scalar=1, op=Op.add)
                    arr[0] = t0[:]
                    arr[1] = t1a[:]
                wxy = {}
                for i in range(2):
                    for j in range(2):
                        t = wp.tile([128, SB], F32, tag=f"wxy{i}{j}")
                        nc.vector.tensor_tensor(out=t[:], in0=wx[i], in1=wy[j], op=Op.mult)
                        wxy[(i, j)] = t[:]
                off_all = iw.tile([128, 8, SB], I16, tag="off")
                hiw = iw.tile([128, 8, SB, 2], BF16, tag="hiw")
                for k, (kx, ky, kz) in enumerate(corners):
                    nc.vector.tensor_tensor(out=t1[:], in0=ax[kx], in1=ay[ky], op=Op.bitwise_xor)
                    nc.vector.tensor_tensor(out=t2[:], in0=t1[:], in1=az[kz], op=Op.bitwise_xor)
                    nc.vector.tensor_single_scalar(out=t1[:], in_=t2[:], scalar=chunk - 1, op=Op.bitwise_and)
                    nc.vector.tensor_copy(out=off_all[:, k, :], in_=t1[:])
                    nc.vector.tensor_scalar(out=t2[:], in0=t2[:], scalar1=lc,
                                            scalar2=15, op0=Op.logical_shift_right, op1=Op.bitwise_and)
                    nc.vector.tensor_copy(out=hiw[:, k, :, 0], in_=t2[:])
                    nc.vector.tensor_tensor(out=hiw[:, k, :, 1], in0=wxy[(kx, ky)], in1=wz[kz], op=Op.mult)
                # round-trip to DRAM: [g][k][q][s][t] (write deferred, see above)
                scr = dp.tile([8, 8, 16, SB, 2], BF16, tag="scr")
                return off_all, scr, hiw, "hashed"

            def prep_flush(pre):
                _, scr, hiw, kind = pre
                if kind == "hashed":
                    for k in range(8):
                        w_ap = AP(scr[:].tensor, scr[:].offset + k * (16 * SB * 2),
                                  [[8 * 16 * SB * 2, 8], [SB * 2, 16], [1, SB * 2]])
                        nc.sync.dma_start(out=w_ap, in_=hiw[:, k, :, :])
                else:
                    hb3 = 16 * (SB // 2) * 3
                    for j in range(4):
                        for h in range(2):
                            w_ap = AP(scr[:].tensor,
                                      scr[:].offset + (j * 2 + h) * hb3,
                                      [[8 * hb3, 8], [(SB // 2) * 3, 16], [1, (SB // 2) * 3]])
                            nc.sync.dma_start(
                                out=w_ap,
                                in_=hiw[:, j, h * (SB // 2):(h + 1) * (SB // 2), :])

            def corners_hashed(LV, b, tab, pre):
                off_all, scr = pre[0], pre[1]
                chunk = LV["chunk"]
                psum = psp.tile([8, NB * 2], F32, tag="psum")
                for k in range(8):
                    hwb = bcp.tile([128, 16, SB, 2], BF16, tag="bc")
                    r_ap = AP(scr[:].tensor, scr[:].offset + k * (16 * SB * 2),
                              [[8 * 16 * SB * 2, 8], [0, 16], [1, 16 * SB * 2]])
                    nc.sync.dma_start(out=hwb[:], in_=r_ap)
                    A = amp.tile([128, NB], BF16, tag="A")
                    hb = hwb[:]
                    hi_v = AP(hb.tensor, hb.offset, [list(hb.ap[0]), [2, NB]])
                    w_v = AP(hb.tensor, hb.offset + 1, [list(hb.ap[0]), [2, NB]])
                    nc.vector.tensor_tensor(out=A[:], in0=hi_v,
                                            in1=qvb[:, 0:1].to_broadcast([128, NB]),
                                            op=Op.is_equal)
                    nc.vector.tensor_tensor(out=A[:], in0=A[:], in1=w_v, op=Op.mult)
                    val = gp.tile([128, NB, 2], BF16, tag="val")
                    nc.gpsimd.ap_gather(
                        out_ap=val[:], in_ap=tab[:], idxs_ap=off_all[:, k, :],
                        channels=128, num_elems=chunk, d=2, num_idxs=NB)
                    av = A[:]
                    a_v = AP(av.tensor, av.offset, [list(av.ap[0]), [1, SB], [SB, 16], [0, 2]])
                    nc.vector.tensor_tensor(out=val[:], in0=val[:], in1=a_v, op=Op.mult)
                    for c4 in range(NB // 256):
                        nc.tensor.matmul(
                            out=psum[:, c4 * 512:(c4 + 1) * 512],
                            lhsT=smat_t[:],
                            rhs=val[:, c4 * 256:(c4 + 1) * 256, :].rearrange("p a b -> p (a b)"),
                            start=(k == 0), stop=(k == 7))
                return psum

            def corners_dense(LV, b, tab, pre):
                off_d, scr = pre[0], pre[1]
                chunkp = LV["ne"]
                psum = psp.tile([8, NB * 2], F32, tag="psum")
                hb3 = 16 * HB * 3
                for j in range(4):
                    for h in range(2):
                        hw3 = bcp.tile([128, 16, HB, 3], BF16, tag="bc")
                        r_ap = AP(scr[:].tensor, scr[:].offset + (j * 2 + h) * hb3,
                                  [[8 * hb3, 8], [0, 16], [1, hb3]])
                        nc.sync.dma_start(out=hw3[:], in_=r_ap)
                        hb = hw3[:]
                        A = amp.tile([128, 2, 16 * HB], BF16, tag="A")
                        hi_v = AP(hb.tensor, hb.offset, [list(hb.ap[0]), [3, 16 * HB]])
                        w0_v = AP(hb.tensor, hb.offset + 1, [list(hb.ap[0]), [3, 16 * HB]])
                        w1_v = AP(hb.tensor, hb.offset + 2, [list(hb.ap[0]), [3, 16 * HB]])
                        nc.vector.tensor_tensor(out=A[:, 0, :], in0=hi_v,
                                                in1=qvb[:, 0:1].to_broadcast([128, 16 * HB]),
                                                op=Op.is_equal)
                        nc.vector.tensor_tensor(out=A[:, 1, :], in0=A[:, 0, :], in1=w1_v, op=Op.mult)
                        nc.vector.tensor_tensor(out=A[:, 0, :], in0=A[:, 0, :], in1=w0_v, op=Op.mult)
                        val4 = gp.tile([128, 16 * HB, 2, 2], BF16, tag="val")
                        nc.gpsimd.ap_gather(
                            out_ap=val4[:], in_ap=tab[:],
                            idxs_ap=off_d[:, j, h * HB:(h + 1) * HB],
                            channels=128, num_elems=chunkp, d=4, num_idxs=16 * HB)
                        av = A[:]
                        for w in range(2):
                            a_v = AP(av.tensor, av.offset + w * (16 * HB),
                                     [list(av.ap[0]), [1, HB], [HB, 16], [0, 2]])
                            nc.vector.tensor_tensor(out=val4[:, :, w, :],
                                                    in0=val4[:, :, w, :], in1=a_v, op=Op.mult)
                        for w in range(2):
                            for c4 in range(4):
                                nc.tensor.matmul(
                                    out=psum[:, h * 2048 + c4 * 512:h * 2048 + (c4 + 1) * 512],
                                    lhsT=smat_t[:],
                                    rhs=val4[:, c4 * 256:(c4 + 1) * 256, w, :],
                                    start=(j == 0 and w == 0), stop=(j == 3 and w == 1))
                return psum

            def flush(LV, b, psum):
                for h in range(2):
                    outsb = wp.tile([8, NB], F16, tag="outsb")
                    nc.scalar.copy(out=outsb[:], in_=psum[:, h * NB:(h + 1) * NB])
                    o_ap = AP(out[:].tensor, (b * NB + h * (NB // 2)) * 32 + 2 * LV["l"],
                              [[NG * 32, 8], [32, NB // 2], [1, 2]])
                    nc.sync.dma_start(out=o_ap, in_=outsb[:].rearrange("p (a b) -> p a b", b=2))

            def stage_pa():
                # one 16-partition group per hashed level; every level covers
                # its table fully (hsize = 16 * chunk)
                chunk = PA_LEVELS[0]["chunk"]
                tab = tabp.tile([128, chunk, 2], BF16, tag="tab")
                for g, LV in enumerate(PA_LEVELS):
                    src = AP(emb[:].tensor, LV["off"] * 2,
                             [[chunk * 2, 16], [1, chunk * 2]])
                    nc.sync.dma_start(
                        out=tab[16 * g:16 * (g + 1)].rearrange("p a b -> p (a b)"),
                        in_=src)
                return tab

            def flush_pa(b, psum):
                l0 = PA_LEVELS[0]["l"]
                for h in range(2):
                    outsb = wp.tile([8, NB], F16, tag="outsb")
                    nc.scalar.copy(out=outsb[:], in_=psum[:, h * NB:(h + 1) * NB])
                    o_ap = AP(out[:].tensor, (b * NB + h * (NB // 2)) * 32 + 2 * l0,
                              [[2, 8], [32, NB // 2], [1, 2]])
                    nc.sync.dma_start(out=o_ap, in_=outsb[:].rearrange("p (a b) -> p a b", b=2))

            jobs = []
            for _ in range(KREP):
                if PA_LEVELS:
                    jobs += [("pa", b) for b in range(NPC // NB)]
                jobs += [(LV, b) for LV in OLD_LEVELS for b in range(NBATCH)]

            def issue_prep(job):
                LV, b = job
                if LV == "pa":
                    return prep(PA_LEVELS[0], b, pa=True)
                return prep(LV, b)

            tab_cur = None
            pre_cur = None
            for i, (LV, b) in enumerate(jobs):
                if b == 0:
                    tab_cur = stage_pa() if LV == "pa" else stage(LV)
                if pre_cur is None:
                    pre_cur = issue_prep((LV, b))
                    prep_flush(pre_cur)
                # issue next batch's index/mask prep before this batch's
                # corner pipeline so DVE work hides under the gather stream;
                # its DRAM write is issued AFTER this batch's corner DMAs so a
                # DVE-sem wait can't head-of-line-block the bcast reads
                pre_nxt = issue_prep(jobs[i + 1]) if i + 1 < len(jobs) else None
                if LV == "pa":
                    psum = corners_hashed(PA_LEVELS[0], b, tab_cur, pre_cur)
                elif LV["hashed"]:
                    psum = corners_hashed(LV, b, tab_cur, pre_cur)
                else:
                    psum = corners_dense(LV, b, tab_cur, pre_cur)
                if pre_nxt is not None:
                    prep_flush(pre_nxt)
                if LV == "pa":
                    flush_pa(b, psum)
                else:
                    flush(LV, b, psum)
                pre_cur = pre_nxt
    nc.compile()
    return nc


_RUNNER = None          # (jitted fn, in_names, out_names, out_avals, mesh, n_params)
_CONST_CACHE = None     # device arrays for replicated constants (smat, qvec)
_EMB_CACHE = None       # (source bytes fingerprint array, device array)
_MEANS_CACHE = None     # (source array copy, device array)
_PREV_OUTS = None       # last call's device outputs, donated back next call


def _make_runner(nc):
    """Build the jitted shard_map executable ONCE (mirrors
    bass2jax.run_bass_via_pjrt's multi-core path, but cached so repeat
    kernel() calls skip retracing and recompilation)."""
    import jax
    from jax.experimental.shard_map import shard_map
    from jax.sharding import Mesh, PartitionSpec
    from concourse import bass2jax, mybir as mb

    bass2jax.install_neuronx_cc_hook()

    in_names, out_names, out_avals = [], [], []
    partition_name = nc.partition_id_tensor.name if nc.partition_id_tensor else None
    for alloc in nc.m.functions[0].allocations:
        if not isinstance(alloc, mb.MemoryLocationSet):
            continue
        name = alloc.memorylocations[0].name
        if alloc.kind == "ExternalInput":
            if name != partition_name:
                in_names.append(name)
        elif alloc.kind == "ExternalOutput":
            out_names.append(name)
            out_avals.append(jax.core.ShapedArray(
                tuple(alloc.tensor_shape), mybir.dt.np(alloc.dtype)))
    n_params = len(in_names)
    n_outs = len(out_avals)
    all_in = list(in_names) + list(out_names)
    if partition_name is not None:
        all_in.append(partition_name)

    def _body(*args):
        operands = list(args)
        if partition_name is not None:
            operands.append(bass2jax.partition_id_tensor())
        outs = bass2jax._bass_exec_p.bind(
            *operands,
            out_avals=tuple(out_avals),
            in_names=tuple(all_in),
            out_names=tuple(out_names),
            lowering_input_output_aliases=(),
            sim_require_finite=True,
            sim_require_nnan=True,
            nc=nc,
        )
        return tuple(outs)

    devices = jax.devices()[:NCORES]
    mesh = Mesh(np.asarray(devices), ("core",))
    donate = tuple(range(n_params, n_params + n_outs))
    in_specs = (PartitionSpec("core"),) * (n_params + n_outs)
    out_specs = (PartitionSpec("core"),) * n_outs
    fn = jax.jit(
        shard_map(_body, mesh=mesh, in_specs=in_specs, out_specs=out_specs,
                  check_rep=False),
        donate_argnums=donate, keep_unused=True)
    return fn, in_names, out_names, out_avals, mesh, n_params


def _dev_put(arr, mesh):
    import jax
    from jax.sharding import NamedSharding, PartitionSpec
    return jax.device_put(arr, NamedSharding(mesh, PartitionSpec("core")))


def kernel(input_means: np.ndarray, embeddings: np.ndarray) -> np.ndarray:
    global _NC_CACHE, _RUNNER, _CONST_CACHE, _EMB_CACHE
    import jax
    import jax.numpy as jnp
    from jax.sharding import NamedSharding, PartitionSpec

    if _NC_CACHE is None:
        _NC_CACHE = _build()
    nc = _NC_CACHE
    if _RUNNER is None:
        _RUNNER = _make_runner(nc)
    fn, in_names, out_names, out_avals, mesh, n_params = _RUNNER

    if _CONST_CACHE is None:
        smat = np.zeros((128, 8), dtype=ml_dtypes.bfloat16)
        for g in range(8):
            smat[16 * g:16 * (g + 1), g] = 1.0
        qvec = (np.arange(128, dtype=np.float32) % 16).reshape(128, 1)
        pvec = np.zeros((128, 1), dtype=np.float32)
        for g, LV in enumerate(PA_LEVELS[:8]):
            pvec[16 * g:16 * (g + 1)] = np.float32(LV["scale"])
        _CONST_CACHE = {
            "smat": _dev_put(np.tile(smat, (NCORES, 1)), mesh),
            "qvec": _dev_put(np.tile(qvec, (NCORES, 1)), mesh),
            "pvec": _dev_put(np.tile(pvec, (NCORES, 1)), mesh),
        }

    # embeddings: bf16-convert + replicate to all cores; cache device copy
    # across calls as long as the source bytes are unchanged.
    emb_f32 = np.ascontiguousarray(embeddings, dtype=np.float32)
    if _EMB_CACHE is None or not np.array_equal(_EMB_CACHE[0], emb_f32):
        emb_bf = emb_f32.astype(ml_dtypes.bfloat16)
        emb_rep = np.broadcast_to(
            emb_bf[None], (NCORES,) + emb_bf.shape).reshape(
                NCORES * emb_bf.shape[0], emb_bf.shape[1])
        _EMB_CACHE = (emb_f32.copy(), _dev_put(np.ascontiguousarray(emb_rep), mesh))
    emb_dev = _EMB_CACHE[1]

    global _MEANS_CACHE
    means = np.ascontiguousarray(input_means, dtype=np.float32)
    if _MEANS_CACHE is None or not np.array_equal(_MEANS_CACHE[0], means):
        _MEANS_CACHE = (means.copy(), _dev_put(means, mesh))
    host_in = {"means": _MEANS_CACHE[1], "emb": emb_dev,
               "smat": _CONST_CACHE["smat"], "qvec": _CONST_CACHE["qvec"],
               "pvec": _CONST_CACHE["pvec"]}
    args = [host_in[name] if not isinstance(host_in[name], np.ndarray)
            else _dev_put(host_in[name], mesh) for name in in_names]
    global _PREV_OUTS
    if _PREV_OUTS is None:
        sh = NamedSharding(mesh, PartitionSpec("core"))
        donate = [jnp.zeros((NCORES * a.shape[0],) + tuple(a.shape[1:]), a.dtype,
                            device=sh) for a in out_avals]
    else:
        donate = _PREV_OUTS
    outs = fn(*args, *donate)
    out_map = dict(zip(out_names, outs))
    res = np.asarray(out_map["out"]).astype(np.float32)
    # outputs are fully overwritten by the kernel, so recycle the device
    # buffers as next call's donated output operands (skips zero-fill).
    _PREV_OUTS = list(outs)
    return res

